# revision 48
# baseline (speedup 1.0000x reference)
"""Trainium2 Bass kernel for nn_Attention_86663850099018.

Math (per batch b, reference semantics):
    xn = x_b / ||x_b rows||                      # (N, E) row-normalized
    S  = xn @ xn.T                               # (N, N) cosine scores, symmetric
    P  = softmax(S, axis=1)                      # row softmax over keys
    U  = P @ h_b                                 # (N, H)
    out = U / frob_norm(U over all batches)      # reference's H* factor cancels

S is symmetric and bounded in [-1,1], so softmax needs no max subtraction
and E = exp(S) stays symmetric: the row block computed in [i-part, j-free]
layout doubles as the stationary operand of the second matmul — no score
transpose anywhere. Rows are relabeled p-major (row = p*16 + t) so every
DRAM<->SBUF transfer is contiguous per partition.

Sharding: data-parallel over batch B=8, one batch per NeuronCore; the
global Frobenius norm needs one 4-byte AllReduce.

Schedule notes (from trace analysis):
  - x loads first on both HWDGE rings in 8 fine chunks; h queues behind
    x on the same rings so it cannot steal HBM bandwidth from the
    phase-A-gating x load.
  - Dummy matmuls at t=0 warm the PE HAM clock gate (transposes don't
    count as PE-busy for HAM); their PSUM reader is virtual-delayed so
    it cannot block the DVE FIFO.
  - tile_wait_until virtual ready-times pin the phase-0 chunk pipeline
    and the h fp16 casts where the scheduler's contention-blind DMA
    model would otherwise head-of-line-block the DVE/ACT FIFOs.
  - A dummy AllReduce right after make_identity forces the gpsimd CC
    library load (~11.5us) to happen under the input DMA instead of in
    front of the real collective at the tail.  After that point gpsimd
    must issue NO pool-compute ops (dma_start/collective only).
  - zinv is computed per row block so phase-B PSUM drains never wait on
    the full phase A; ssq comes straight off PSUM via ACT Square with
    scale=zinv, off the DVE critical path.
  - Tail: AllReduce(add) of the per-core sum of squares, sqrt/recip,
    PE-broadcast of the scalar, then scale+writeback chunks pipelined
    on all three DMA rings.
"""

import numpy as np

N, B, E, H = 2048, 8, 256, 512
P = 128
NT = N // P      # 16 row tiles
EC = E // P      # 2 contraction chunks for scores
SF = 512         # matmul free-dim chunk
FC = N // SF     # 4 score chunks per row block
XCH = 8          # x input DMA chunks
HCH = 4          # h input DMA chunks
TCH = NT // XCH  # row tiles per x chunk
NCORES = 8

_CACHE = {}


def _build():
    import concourse.mybir as mybir
    import concourse.tile as tile
    from concourse import bacc
    from concourse.masks import make_identity

    f32 = mybir.dt.float32
    f16 = mybir.dt.float16
    AF = mybir.ActivationFunctionType
    ALU = mybir.AluOpType
    AX = mybir.AxisListType

    nc = bacc.Bacc("TRN2", target_bir_lowering=False, debug=False, num_devices=NCORES)

    x_d = nc.dram_tensor("x", [N, E], f32, kind="ExternalInput").ap()
    h_d = nc.dram_tensor("h", [N, H], f32, kind="ExternalInput").ap()
    o_d = nc.dram_tensor("out", [N, H], f32, kind="ExternalOutput").ap()

    # p-major row relabeling: row = p*NT + t  (16 KiB contiguous / partition)
    x_pt = x_d.rearrange("(p t) e -> p t e", t=NT)
    h_pt = h_d.rearrange("(p t) e -> p t e", t=NT)
    o_pt = o_d.rearrange("(p t) e -> p t e", t=NT)

    with tile.TileContext(nc) as tc:
        with (
            tc.tile_pool(name="const", bufs=1) as constp,
            tc.tile_pool(name="eexpp", bufs=1) as eexpp,
            tc.tile_pool(name="hp", bufs=1) as hp,
            tc.tile_pool(name="zp", bufs=1) as zp,
            tc.tile_pool(name="dramp", bufs=1, space="DRAM") as dramp,
        ):
            ident = constp.tile([P, P], f16)
            ones = constp.tile([P, 1], f32)
            ones8 = constp.tile([NCORES, P], f32)
            wsrc = constp.tile([P, SF], f16)

            eexp = eexpp.tile([P, NT, N], f16)        # 64 KiB/partition
            h_sb = hp.tile([P, NT, H], f16)           # 16 KiB/partition

            zsum = zp.tile([P, NT], f32)
            zinv = zp.tile([P, NT], f32)
            ssqraw = zp.tile([P, NT], f32)
            ssqcol = zp.tile([P, 1], f32)

            # ---- PE warmup: keep the HAM clock gate busy from t=0 until
            # the first x chunk is normalized+transposed. PE transposes do
            # NOT count as HAM activity, so sprinkle MMs continue through
            # the phase-0 transpose window (emitted in the chunk loop).
            nc.vector.memset(wsrc[:], 0.0)
            nc.vector.memset(ones[:], 1.0)
            nc.vector.memset(ones8[:], 1.0)
            wrd = zp.tile([1, 1], f32)
            with tc.tile_pool(name="psW", bufs=1, space="PSUM") as psW:
                WMM = 10
                wps = psW.tile([P, SF], f32)
                for k in range(WMM):
                    nc.tensor.matmul(
                        wps[:], wsrc[:, :P], wsrc[:],
                        start=(k == 0), stop=(k == WMM - 1),
                    )
                # reader keeps the warmup MMs alive; virtual-delay it so it
                # cannot head-of-line-block the DVE FIFO during phase 0
                # (but not so late that psW's bank reuse by psT stalls)
                with tc.tile_wait_until(0.012):
                    nc.vector.tensor_copy(wrd[:], wps[:1, :1])

            # preload the sqrt table set before the phase-0 norms need it
            sqrtpre0 = zp.tile([1, 1], f32)
            nc.scalar.activation(sqrtpre0[:], ones[:1, :], AF.Sqrt)

            make_identity(nc, ident[:])

            # ---- dummy collective: load the CC library early (under DMA).
            # Reads ident so it cannot be scheduled before make_identity's
            # gpsimd pool-compute ops. No gpsimd pool-compute after this.
            cc_in = dramp.tile([1, 1], f32)
            cc_out = dramp.tile([NCORES, 1], f32)
            cc_in0 = dramp.tile([1, 1], f32)
            cc_out0 = dramp.tile([1, 1], f32)
            warm_cc = zp.tile([1, 1], f32)
            nc.vector.tensor_copy(warm_cc[:], ident[:1, :1])
            nc.gpsimd.dma_start(cc_in0[:], warm_cc[:])
            nc.gpsimd.collective_compute(
                "AllReduce",
                ALU.add,
                replica_groups=[list(range(NCORES))],
                ins=[cc_in0.opt()],
                outs=[cc_out0.opt()],
            )
            nc.gpsimd.dma_start(warm_cc[:], cc_out0[:])

            # ---------------- phase 0: load, normalize, transpose ----------
            with tc.tile_pool(name="xntp", bufs=1) as xntp:
                x_all = xntp.tile([P, NT, E], f32)    # 16 KiB/partition
                xnt = xntp.tile([P, EC, N], f16)      # 8 KiB/partition
                ssq_all = xntp.tile([P, NT], f32)
                lnssq = xntp.tile([P, NT], f32)
                invn = xntp.tile([P, NT], f32)
                htmp = xntp.tile([P, NT, H], f32)     # 32 KiB/partition

                # psB opens before psA/psT so early phase-B groups can
                # interleave into PE gaps while ACT paces phase A.
                psB_ctx = tc.tile_pool(name="psB", bufs=4, space="PSUM")
                psB = psB_ctx.__enter__()

                with (
                    tc.tile_pool(name="ph0", bufs=3) as ph0,
                    tc.tile_pool(name="psT", bufs=2, space="PSUM") as psT,
                ):
                    # x first on both HWDGE rings (phase A gates on it); h
                    # queues BEHIND x on the same rings so it cannot steal
                    # HBM bandwidth from x. gpsimd ring stays free for the
                    # dummy collective.
                    # ALL input triggers on the sync ring: scalar-ring DMA
                    # triggers occupy the ACT engine FIFO (~1us each) and
                    # delay the phase-0 squares. One HWDGE ring sustains
                    # ~286GB/s, and chunk-sequential arrival starts the
                    # normalize/transpose pipeline at ~9us.
                    for ch in range(XCH):
                        t0 = ch * TCH
                        nc.sync.dma_start(
                            x_all[:, t0 : t0 + TCH, :], x_pt[:, t0 : t0 + TCH, :]
                        )
                    for ch in range(HCH):
                        t0 = ch * (NT // HCH)
                        nc.sync.dma_start(
                            htmp[:, t0 : t0 + NT // HCH, :],
                            h_pt[:, t0 : t0 + NT // HCH, :],
                        )
                    # Per-chunk virtual ready-times force the scheduler to
                    # keep each chunk's norm->scale->transpose chain
                    # together (its DMA model ignores ring contention and
                    # otherwise batches same-kind ops, which serializes the
                    # whole pipeline behind the last x chunk).
                    for ch in range(XCH):
                        t0 = ch * TCH
                        with tc.tile_wait_until(0.0085 + 0.0009 * ch):
                            scr = ph0.tile([P, TCH, E], f32, tag="scr")
                            nc.scalar.activation(
                                scr[:], x_all[:, t0 : t0 + TCH, :], AF.Square
                            )
                            nc.vector.tensor_reduce(
                                ssq_all[:, t0 : t0 + TCH],
                                scr[:],
                                axis=AX.X,
                                op=ALU.add,
                            )
                            nc.scalar.activation(
                                lnssq[:, t0 : t0 + TCH],
                                ssq_all[:, t0 : t0 + TCH],
                                AF.Sqrt,
                            )
                            nc.vector.reciprocal(
                                invn[:, t0 : t0 + TCH], lnssq[:, t0 : t0 + TCH]
                            )
                            for t in range(t0, t0 + TCH, 2):
                                pt = psT.tile([P, 2 * EC, P], f16, tag="pt")
                                for u in range(2):
                                    xn = ph0.tile([P, E], f16, tag="xn")
                                    nc.vector.tensor_scalar_mul(
                                        xn[:],
                                        x_all[:, t + u, :],
                                        invn[:, t + u : t + u + 1],
                                    )
                                    for c in range(EC):
                                        nc.tensor.transpose(
                                            pt[:, u * EC + c, :],
                                            xn[:, c * P : (c + 1) * P],
                                            ident[:],
                                        )
                                # pt[u*EC+c, k] -> xnt[c, (t+u)*P + k]; ACT
                                # copy keeps the DVE free for reductions
                                nc.scalar.copy(
                                    xnt[:, :, t * P : (t + 2) * P].rearrange(
                                        "p c (u k) -> p u c k", k=P
                                    ),
                                    pt[:].rearrange("p (u c) k -> p u c k", c=EC),
                                )
                            # HAM feeder: transposes don't count as PE-busy
                            # and a token MM is below the busy threshold, so
                            # burn a real 2xN=512 MM burst per chunk (~28%
                            # duty) to hold the clock gate at 2.4GHz
                            hamt = psT.tile([P, SF], f32, tag="hamf")
                            nc.tensor.matmul(
                                hamt[:], wsrc[:, :P], wsrc[:],
                                start=True, stop=False,
                            )
                            nc.tensor.matmul(
                                hamt[:], wsrc[:, :P], wsrc[:],
                                start=False, stop=True,
                            )
                            nc.vector.tensor_copy(wrd[:], hamt[:1, :1])

                # ---------------- phase A: scores + exp ---------------------
                # (psT/ph0 closed: their banks go to psA)
                # The h fp16 casts ride INSIDE this loop so they land late
                # in the DVE FIFO: emitted up front they head-of-line block
                # the phase-0 reductions while waiting for h's DMA.
                with tc.tile_pool(name="psA", bufs=2, space="PSUM") as psA:
                    for i in range(NT):
                        for half in range(2):
                            ps = psA.tile([P, 2, SF], f32, tag="psA")
                            for c in range(EC):
                                for q in range(2):
                                    jc = half * 2 + q
                                    nc.tensor.matmul(
                                        ps[:, q, :],
                                        xnt[:, c, i * P : (i + 1) * P],
                                        xnt[:, c, jc * SF : (jc + 1) * SF],
                                        start=(c == 0),
                                        stop=(c == EC - 1),
                                    )
                            nc.scalar.activation(
                                eexp[:, i, half * 2 * SF : (half + 1) * 2 * SF],
                                ps[:].rearrange("p a b -> p (a b)"),
                                AF.Exp,
                            )
                        nc.vector.tensor_reduce(
                            zsum[:, i : i + 1], eexp[:, i, :], axis=AX.X, op=ALU.add
                        )
                        nc.vector.reciprocal(
                            zinv[:, i : i + 1], zsum[:, i : i + 1]
                        )
                        # h fp16 rounding, one row block per iteration.
                        # tile_wait_until keeps the scheduler from hoisting
                        # these to the head of the DVE FIFO where they would
                        # head-of-line-block the phase-0 reductions while
                        # waiting on the h DMA.
                        with tc.tile_wait_until(0.018 + 0.0015 * i):
                            nc.vector.tensor_copy(h_sb[:, i, :], htmp[:, i, :])

            # ---------------- phase B: U = exp(S) @ h -----------------------
            # psT/psA/ph0/xntp freed; open more PSUM for deep pipelining.
            psB2_ctx = tc.tile_pool(name="psB2", bufs=3, space="PSUM")
            psB2 = psB2_ctx.__enter__()

            with (
                tc.tile_pool(name="outp", bufs=1) as outp,
                tc.tile_pool(name="tailp", bufs=2) as tailp,
            ):
                out_sb = outp.tile([P, NT, H], f32)   # 32 KiB/partition

                # preload the sqrt table set (hidden under phase-B PE work)
                sqpre = tailp.tile([1, 1], f32, tag="sqpre")
                nc.scalar.activation(sqpre[:], zsum[:1, :1], AF.Sqrt)

                for j in range(NT):
                    pool = psB if (j < 4 or j % 2 == 0) else psB2
                    ps = pool.tile([P, H], f32, tag="psB")
                    for i in range(NT):
                        nc.tensor.matmul(
                            ps[:],
                            eexp[:, i, j * P : (j + 1) * P],
                            h_sb[:, i, :],
                            start=(i == 0),
                            stop=(i == NT - 1),
                        )
                    # ssq contribution straight off PSUM: (zinv*ps)^2
                    sqs = tailp.tile([P, H], f32, tag="sqs")
                    nc.scalar.activation(
                        sqs[:],
                        ps[:],
                        AF.Square,
                        scale=zinv[:, j : j + 1],
                        accum_out=ssqraw[:, j : j + 1],
                    )
                    # zinv-scaled U to SBUF (DVE); the last two blocks are
                    # deferred below the collective trigger so the DVE FIFO
                    # cannot delay the global-norm critical path
                    if j < NT - 2:
                        nc.vector.tensor_scalar_mul(
                            out_sb[:, j, :], ps[:], zinv[:, j : j + 1]
                        )
                        pslast = {}
                    elif j == NT - 2:
                        pslast = {j: ps}
                    else:
                        pslast[j] = ps

                # ---------------- tail: global norm + writeback -------------
                nc.vector.tensor_reduce(ssqcol[:], ssqraw[:], axis=AX.X, op=ALU.add)

                psS_ctx = tc.tile_pool(name="psS", bufs=1, space="PSUM")
                psS = psS_ctx.__enter__()
                psSt = psS.tile([P, 1], f32, tag="psSt")
                nc.tensor.matmul(
                    psSt[:1, :], ones[:], ssqcol[:], start=True, stop=True
                )
                ss11 = tailp.tile([1, 1], f32, tag="ss11")
                nc.scalar.copy(ss11[:], psSt[:1, :])

                nc.scalar.dma_start(cc_in[:], ss11[:])
                nc.gpsimd.collective_compute(
                    "AllGather",
                    ALU.bypass,
                    replica_groups=[list(range(NCORES))],
                    ins=[cc_in.opt()],
                    outs=[cc_out.opt()],
                )
                agg = tailp.tile([NCORES, 1], f32, tag="agg")
                nc.sync.dma_start(agg[:], cc_out[:])

                # deferred last two U blocks ride under the collective
                for j, ps in pslast.items():
                    nc.vector.tensor_scalar_mul(
                        out_sb[:, j, :], ps[:], zinv[:, j : j + 1]
                    )

                # one matmul sums the 8 gathered scalars AND broadcasts to
                # all partitions: out[p] = sum_c agg[c]
                nc.tensor.matmul(psSt[:], ones8[:], agg[:], start=True, stop=True)
                lnt = tailp.tile([P, 1], f32, tag="lnt")
                gbc = tailp.tile([P, 1], f32, tag="gbc")
                nc.scalar.activation(lnt[:], psSt[:], AF.Sqrt)
                nc.vector.reciprocal(gbc[:], lnt[:])

                # uniform 1/gnorm scale on DVE; writeback on all three rings
                OCH = 4
                engs = [nc.sync, nc.scalar, nc.gpsimd, nc.sync]
                for ch in range(OCH):
                    j0 = ch * (NT // OCH)
                    blk = out_sb[:, j0 : j0 + NT // OCH, :]
                    nc.vector.tensor_scalar_mul(blk, blk, gbc[:])
                    engs[ch].dma_start(o_pt[:, j0 : j0 + NT // OCH, :], blk)
                psS_ctx.__exit__(None, None, None)
            psB2_ctx.__exit__(None, None, None)
            psB_ctx.__exit__(None, None, None)

    nc.compile()
    return nc


def _get_nc():
    if "nc" not in _CACHE:
        _CACHE["nc"] = _build()
    return _CACHE["nc"]


def _in_maps(x, h):
    return [
        {
            "x": np.ascontiguousarray(x[:, c, :]),
            "h": np.ascontiguousarray(h[:, c, :]),
        }
        for c in range(NCORES)
    ]


def kernel(x, h):
    from concourse.bass_utils import run_bass_kernel_spmd

    x = np.asarray(x, dtype=np.float32)
    h = np.asarray(h, dtype=np.float32)
    assert x.shape == (N, B, E) and h.shape == (N, B, H)

    nc = _get_nc()
    res = run_bass_kernel_spmd(nc, _in_maps(x, h), core_ids=list(range(NCORES)))
    out = np.empty((N, B, H), dtype=np.float32)
    for c in range(NCORES):
        out[:, c, :] = res.results[c]["out"]
    return out


# Exposed for test.py: run once with tracing to get hardware exec time.
def run_traced(x, h):
    import os
    import shutil

    from concourse.bass_utils import run_bass_kernel_spmd

    x = np.asarray(x, dtype=np.float32)
    h = np.asarray(h, dtype=np.float32)
    nc = _get_nc()
    tdir = "/root/problem/trace_out"
    shutil.rmtree(tdir, ignore_errors=True)
    os.makedirs(tdir, exist_ok=True)
    res = run_bass_kernel_spmd(
        nc, _in_maps(x, h), core_ids=list(range(NCORES)), trace=True, tmpdir=tdir
    )
    out = np.empty((N, B, H), dtype=np.float32)
    for c in range(NCORES):
        out[:, c, :] = res.results[c]["out"]
    return out, res
